# revision 16
# baseline (speedup 1.0000x reference)
"""Trainium2 Bass kernel for nn_Decoder: bit-unpack 23x22-bit codes per batch
row, gather 1536B fp16 table rows by index, sign-flip about 0.5, emit the data
rows (rows 19:67 of each [2, 126, 128] plane) as fp16. The constant-0.5 filler
regions and the fp32 upcast are data-independent, so the host materializes
them during unshard (assemble_output) — this cuts device HBM writes from
132MB to 25MB per core.

Gathers use per-code indirect DMAs: one offset per partition is the only
form the DMA_INDIRECT ucode supports (multi-offset APs stream contiguously
from offset[p,0] or crash the exec unit). At ~1.4us serialized descriptor-gen
per instruction on the GpSimd engine, the 184 gathers per core (8 groups x 23
codes) are the critical path (~260us of the ~290us exec).

Sharding: data-parallel over batch across 8 NeuronCores (1024 rows each);
the lookup table is replicated on every core.

Self-contained: hardcodes all shapes; no imports from the problem directory.
"""

import numpy as np

import concourse.bacc as bacc
import concourse.bass as bass
import concourse.mybir as mybir
import concourse.tile as tile

BATCH = 8192
XCOLS = 512
NCODE = 23
NBITS = 22
L = 131072
ROW = 2 * 48 * 8     # 768 fp16 = 1536B per table row
NCORES = 8
BC = BATCH // NCORES
P = 128
GROUPS = BC // P
DROW = 2 * 48 * 128  # 12288

f16 = mybir.dt.float16
f32 = mybir.dt.float32
i32 = mybir.dt.int32

N_SWDGE_QUEUES = 2


def _code_map(c):
    if c < 7:
        return 0, 4, c * 8
    if c < 14:
        return 4, 4, (c - 7) * 8 + 4
    return 0, 8, (c - 7) * 8


def build_module():
    nc = bacc.Bacc(
        "TRN2", target_bir_lowering=False, debug=False,
        num_swdge_queues=N_SWDGE_QUEUES,
    )
    x_t = nc.dram_tensor("x", [BC, XCOLS], i32, kind="ExternalInput")
    tab_t = nc.dram_tensor("table", [L, ROW], f16, kind="ExternalInput")
    w_t = nc.dram_tensor("w", [P, NCODE * NBITS], f32, kind="ExternalInput")
    out_t = nc.dram_tensor("out", [BC, DROW], f16, kind="ExternalOutput")

    with tile.TileContext(nc) as tc:
        with (
            tc.tile_pool(name="const", bufs=1) as cpool,
            tc.tile_pool(name="xp", bufs=2) as xpool,
            tc.tile_pool(name="sm", bufs=GROUPS) as spool,
            tc.tile_pool(name="gt", bufs=16) as gpool,
            tc.tile_pool(name="op", bufs=2) as opool,
        ):
            # Fast path for the very first gather: it only needs idx of
            # (group 0, code 0) = bits x[0:128, 6:28]. A 128B/row load plus a
            # 22-wide decode chain gets it ready ~4us before the full group-0
            # decode, pulling the whole gather stream earlier.
            x0a = cpool.tile([P, 32], i32)
            nc.sync.dma_start(x0a[:], x_t[0:P, 0:32])
            w_tile = cpool.tile([P, NCODE * NBITS], f32)
            nc.sync.dma_start(w_tile[:], w_t[:])
            xfa = cpool.tile([P, 32], f32)
            nc.vector.tensor_copy(out=xfa[:], in_=x0a[:])
            proda = cpool.tile([P, NBITS], f32)
            nc.vector.tensor_tensor(
                out=proda[:], in0=xfa[:, 6 : 6 + NBITS], in1=w_tile[:, 0:NBITS],
                op=mybir.AluOpType.mult,
            )
            codes0 = cpool.tile([P, 1], f32)
            nc.vector.tensor_reduce(
                out=codes0[:],
                in_=proda[:].rearrange("n (c a) -> n c a", a=NBITS),
                axis=mybir.AxisListType.X,
                op=mybir.AluOpType.add,
            )
            codes0i = cpool.tile([P, 1], i32)
            nc.vector.tensor_copy(out=codes0i[:], in_=codes0[:])
            idx0 = cpool.tile([P, 1], i32)
            nc.vector.tensor_scalar(
                out=idx0[:], in0=codes0i[:],
                scalar1=L - 1, scalar2=None,
                op0=mybir.AluOpType.bitwise_and,
            )

            idxs, tts, sgs = [], [], []
            for g in range(GROUPS):
                b0 = g * P
                x_tile = xpool.tile([P, XCOLS], i32)
                nc.sync.dma_start(x_tile[:], x_t[b0 : b0 + P, :])
                xf = xpool.tile([P, XCOLS], f32)
                nc.vector.tensor_copy(out=xf[:], in_=x_tile[:])
                prod = xpool.tile([P, NCODE * NBITS], f32)
                nc.vector.tensor_tensor(
                    out=prod[:], in0=xf[:, 6:], in1=w_tile[:],
                    op=mybir.AluOpType.mult,
                )
                codes = spool.tile([P, NCODE], f32, tag="codes")
                nc.vector.tensor_reduce(
                    out=codes[:],
                    in_=prod[:].rearrange("n (c a) -> n c a", a=NBITS),
                    axis=mybir.AxisListType.X,
                    op=mybir.AluOpType.add,
                )
                codesi = spool.tile([P, NCODE], i32, tag="codesi")
                nc.vector.tensor_copy(out=codesi[:], in_=codes[:])
                idx = spool.tile([P, NCODE], i32, tag="idx")
                nc.vector.tensor_scalar(
                    out=idx[:], in0=codesi[:],
                    scalar1=L - 1, scalar2=None,
                    op0=mybir.AluOpType.bitwise_and,
                )
                tt = spool.tile([P, NCODE], f32, tag="tt")
                nc.vector.tensor_scalar(
                    out=tt[:], in0=codes[:],
                    scalar1=float(L), scalar2=None,
                    op0=mybir.AluOpType.is_gt,
                )
                sg = spool.tile([P, NCODE], f32, tag="sg")
                nc.vector.tensor_scalar(
                    out=sg[:], in0=tt[:],
                    scalar1=-2.0, scalar2=1.0,
                    op0=mybir.AluOpType.mult, op1=mybir.AluOpType.add,
                )
                idxs.append(idx); tts.append(tt); sgs.append(sg)

            for g in range(GROUPS):
                b0 = g * P
                idx, tt, sg = idxs[g], tts[g], sgs[g]
                od = opool.tile([P, DROW], f16)
                od4 = od[:].rearrange("n (p k c) -> n p k c", p=2, k=48)
                for c in range(NCODE):
                    gc = gpool.tile([P, ROW], f16)
                    off = idx0[:, 0:1] if (g == 0 and c == 0) else idx[:, c : c + 1]
                    gi = nc.gpsimd.indirect_dma_start(
                        out=gc[:],
                        out_offset=None,
                        in_=tab_t[:],
                        in_offset=bass.IndirectOffsetOnAxis(ap=off, axis=0),
                    )
                    if N_SWDGE_QUEUES > 1 and c % N_SWDGE_QUEUES:
                        gi.ins.queue = f"qPoolDynamic{c % N_SWDGE_QUEUES}"
                    gv = gc[:].rearrange("n (p k c) -> n p k c", p=2, k=48)
                    ch0, wdt, col0 = _code_map(c)
                    if c >= 14:
                        nc.scalar.activation(
                            out=od4[:, :, :, col0 : col0 + wdt],
                            in_=gv[:, :, :, ch0 : ch0 + wdt],
                            func=mybir.ActivationFunctionType.Identity,
                            bias=tt[:, c : c + 1],
                            scale=sg[:, c : c + 1],
                        )
                    else:
                        nc.vector.tensor_scalar(
                            out=od4[:, :, :, col0 : col0 + wdt],
                            in0=gv[:, :, :, ch0 : ch0 + wdt],
                            scalar1=sg[:, c : c + 1],
                            scalar2=tt[:, c : c + 1],
                            op0=mybir.AluOpType.mult,
                            op1=mybir.AluOpType.add,
                        )
                eng = nc.sync if g % 2 == 0 else nc.scalar
                eng.dma_start(out=out_t[b0 : b0 + P, :], in_=od[:])
    nc.compile()
    return nc


def make_weights():
    w = np.tile((2.0 ** np.arange(NBITS)).astype(np.float32), NCODE)
    return np.broadcast_to(w, (P, NCODE * NBITS)).copy()


def make_in_maps(x, table):
    tab = np.ascontiguousarray(table.reshape(L, ROW))
    w = make_weights()
    return [
        {
            "x": np.ascontiguousarray(x[i * BC : (i + 1) * BC]),
            "table": tab,
            "w": w,
        }
        for i in range(NCORES)
    ]


def assemble_output(parts):
    out = np.full((BATCH, 2, 126, 128), 0.5, dtype=np.float32)
    for i, p in enumerate(parts):
        out[i * BC : (i + 1) * BC, :, 19:67, :] = p.reshape(BC, 2, 48, 128)
    return out


_NC_CACHE = None


def _get_module():
    global _NC_CACHE
    if _NC_CACHE is None:
        _NC_CACHE = build_module()
    return _NC_CACHE


def kernel(x: np.ndarray, table: np.ndarray) -> np.ndarray:
    from concourse.bass_utils import run_bass_kernel_spmd

    x = np.asarray(x)
    table = np.asarray(table)
    assert x.shape == (BATCH, XCOLS) and table.shape == (L, 2, 48, 8)
    nc = _get_module()
    res = run_bass_kernel_spmd(nc, make_in_maps(x, table), core_ids=list(range(NCORES)))
    return assemble_output([res.results[i]["out"] for i in range(NCORES)])


# revision 18
# speedup vs baseline: 1.0095x; 1.0095x over previous
"""Trainium2 Bass kernel for nn_Decoder: bit-unpack 23x22-bit codes per batch
row, gather 1536B fp16 table rows by index, sign-flip about 0.5, emit the data
rows (rows 19:67 of each [2, 126, 128] plane) as fp16. The constant-0.5 filler
regions and the fp32 upcast are data-independent, so the host materializes
them during unshard (assemble_output) — this cuts device HBM writes from
132MB to 25MB per core.

Gathers use per-code indirect DMAs: one offset per partition is the only
form the DMA_INDIRECT ucode supports (multi-offset APs stream contiguously
from offset[p,0] or crash the exec unit). At ~1.4us serialized descriptor-gen
per instruction on the GpSimd engine, the 184 gathers per core (8 groups x 23
codes) are the critical path (~260us of the ~290us exec).

Sharding: data-parallel over batch across 8 NeuronCores (1024 rows each);
the lookup table is replicated on every core.

Self-contained: hardcodes all shapes; no imports from the problem directory.
"""

import numpy as np

import concourse.bacc as bacc
import concourse.bass as bass
import concourse.mybir as mybir
import concourse.tile as tile

BATCH = 8192
XCOLS = 512
NCODE = 23
NBITS = 22
L = 131072
ROW = 2 * 48 * 8     # 768 fp16 = 1536B per table row
NCORES = 8
BC = BATCH // NCORES
P = 128
GROUPS = BC // P
DROW = 2 * 48 * 128  # 12288

f16 = mybir.dt.float16
f32 = mybir.dt.float32
i32 = mybir.dt.int32

N_SWDGE_QUEUES = 2


def _code_map(c):
    if c < 7:
        return 0, 4, c * 8
    if c < 14:
        return 4, 4, (c - 7) * 8 + 4
    return 0, 8, (c - 7) * 8


def build_module():
    nc = bacc.Bacc(
        "TRN2", target_bir_lowering=False, debug=False,
        num_swdge_queues=N_SWDGE_QUEUES,
    )
    x_t = nc.dram_tensor("x", [BC, XCOLS], i32, kind="ExternalInput")
    tab_t = nc.dram_tensor("table", [L, ROW], f16, kind="ExternalInput")
    w_t = nc.dram_tensor("w", [P, NCODE * NBITS], f32, kind="ExternalInput")
    out_t = nc.dram_tensor("out", [BC, DROW], f16, kind="ExternalOutput")

    with tile.TileContext(nc) as tc:
        with (
            tc.tile_pool(name="const", bufs=1) as cpool,
            tc.tile_pool(name="xp", bufs=2) as xpool,
            tc.tile_pool(name="sm", bufs=GROUPS) as spool,
            tc.tile_pool(name="gt", bufs=16) as gpool,
            tc.tile_pool(name="op", bufs=2) as opool,
        ):
            # Fast path for the very first gather: it only needs idx of
            # (group 0, code 0) = bits x[0:128, 6:28]. A 128B/row load plus a
            # 22-wide decode chain gets it ready ~4us before the full group-0
            # decode, pulling the whole gather stream earlier.
            x0a = cpool.tile([P, 32], i32)
            nc.sync.dma_start(x0a[:], x_t[0:P, 0:32])
            # dedicated 22-element weight slice so the fast path is not gated
            # on the full 506-element w load's completion
            w0_tile = cpool.tile([P, NBITS], f32)
            nc.sync.dma_start(w0_tile[:], w_t[0:P, 0:NBITS])
            w_tile = cpool.tile([P, NCODE * NBITS], f32)
            nc.sync.dma_start(w_tile[:], w_t[:])
            xfa = cpool.tile([P, 32], f32)
            nc.vector.tensor_copy(out=xfa[:], in_=x0a[:])
            proda = cpool.tile([P, NBITS], f32)
            nc.vector.tensor_tensor(
                out=proda[:], in0=xfa[:, 6 : 6 + NBITS], in1=w0_tile[:],
                op=mybir.AluOpType.mult,
            )
            codes0 = cpool.tile([P, 1], f32)
            nc.vector.tensor_reduce(
                out=codes0[:],
                in_=proda[:].rearrange("n (c a) -> n c a", a=NBITS),
                axis=mybir.AxisListType.X,
                op=mybir.AluOpType.add,
            )
            codes0i = cpool.tile([P, 1], i32)
            nc.vector.tensor_copy(out=codes0i[:], in_=codes0[:])
            idx0 = cpool.tile([P, 1], i32)
            nc.vector.tensor_scalar(
                out=idx0[:], in0=codes0i[:],
                scalar1=L - 1, scalar2=None,
                op0=mybir.AluOpType.bitwise_and,
            )

            idxs, tts, sgs = [], [], []
            for g in range(GROUPS):
                b0 = g * P
                x_tile = xpool.tile([P, XCOLS], i32)
                nc.sync.dma_start(x_tile[:], x_t[b0 : b0 + P, :])
                xf = xpool.tile([P, XCOLS], f32)
                nc.vector.tensor_copy(out=xf[:], in_=x_tile[:])
                prod = xpool.tile([P, NCODE * NBITS], f32)
                nc.vector.tensor_tensor(
                    out=prod[:], in0=xf[:, 6:], in1=w_tile[:],
                    op=mybir.AluOpType.mult,
                )
                codes = spool.tile([P, NCODE], f32, tag="codes")
                nc.vector.tensor_reduce(
                    out=codes[:],
                    in_=prod[:].rearrange("n (c a) -> n c a", a=NBITS),
                    axis=mybir.AxisListType.X,
                    op=mybir.AluOpType.add,
                )
                codesi = spool.tile([P, NCODE], i32, tag="codesi")
                nc.vector.tensor_copy(out=codesi[:], in_=codes[:])
                idx = spool.tile([P, NCODE], i32, tag="idx")
                nc.vector.tensor_scalar(
                    out=idx[:], in0=codesi[:],
                    scalar1=L - 1, scalar2=None,
                    op0=mybir.AluOpType.bitwise_and,
                )
                tt = spool.tile([P, NCODE], f32, tag="tt")
                nc.vector.tensor_scalar(
                    out=tt[:], in0=codes[:],
                    scalar1=float(L), scalar2=None,
                    op0=mybir.AluOpType.is_gt,
                )
                sg = spool.tile([P, NCODE], f32, tag="sg")
                nc.vector.tensor_scalar(
                    out=sg[:], in0=tt[:],
                    scalar1=-2.0, scalar2=1.0,
                    op0=mybir.AluOpType.mult, op1=mybir.AluOpType.add,
                )
                idxs.append(idx); tts.append(tt); sgs.append(sg)

            for g in range(GROUPS):
                b0 = g * P
                idx, tt, sg = idxs[g], tts[g], sgs[g]
                od = opool.tile([P, DROW], f16)
                od4 = od[:].rearrange("n (p k c) -> n p k c", p=2, k=48)
                # last group: wide codes first so the store waits on a fast
                # DVE consumer (~0.45us) instead of an ACT one (~1.1us)
                if g == GROUPS - 1:
                    code_order = list(range(14, NCODE)) + list(range(14))
                else:
                    code_order = list(range(NCODE))
                for c in code_order:
                    gc = gpool.tile([P, ROW], f16)
                    off = idx0[:, 0:1] if (g == 0 and c == 0) else idx[:, c : c + 1]
                    gi = nc.gpsimd.indirect_dma_start(
                        out=gc[:],
                        out_offset=None,
                        in_=tab_t[:],
                        in_offset=bass.IndirectOffsetOnAxis(ap=off, axis=0),
                    )
                    if N_SWDGE_QUEUES > 1 and c % N_SWDGE_QUEUES:
                        gi.ins.queue = f"qPoolDynamic{c % N_SWDGE_QUEUES}"
                    gv = gc[:].rearrange("n (p k c) -> n p k c", p=2, k=48)
                    ch0, wdt, col0 = _code_map(c)
                    if c >= 14:
                        nc.scalar.activation(
                            out=od4[:, :, :, col0 : col0 + wdt],
                            in_=gv[:, :, :, ch0 : ch0 + wdt],
                            func=mybir.ActivationFunctionType.Identity,
                            bias=tt[:, c : c + 1],
                            scale=sg[:, c : c + 1],
                        )
                    else:
                        nc.vector.tensor_scalar(
                            out=od4[:, :, :, col0 : col0 + wdt],
                            in0=gv[:, :, :, ch0 : ch0 + wdt],
                            scalar1=sg[:, c : c + 1],
                            scalar2=tt[:, c : c + 1],
                            op0=mybir.AluOpType.mult,
                            op1=mybir.AluOpType.add,
                        )
                eng = nc.sync if g % 2 == 0 else nc.scalar
                eng.dma_start(out=out_t[b0 : b0 + P, :], in_=od[:])
    nc.compile()
    return nc


def make_weights():
    w = np.tile((2.0 ** np.arange(NBITS)).astype(np.float32), NCODE)
    return np.broadcast_to(w, (P, NCODE * NBITS)).copy()


def make_in_maps(x, table):
    tab = np.ascontiguousarray(table.reshape(L, ROW))
    w = make_weights()
    return [
        {
            "x": np.ascontiguousarray(x[i * BC : (i + 1) * BC]),
            "table": tab,
            "w": w,
        }
        for i in range(NCORES)
    ]


def assemble_output(parts):
    out = np.full((BATCH, 2, 126, 128), 0.5, dtype=np.float32)
    for i, p in enumerate(parts):
        out[i * BC : (i + 1) * BC, :, 19:67, :] = p.reshape(BC, 2, 48, 128)
    return out


_NC_CACHE = None


def _get_module():
    global _NC_CACHE
    if _NC_CACHE is None:
        _NC_CACHE = build_module()
    return _NC_CACHE


def kernel(x: np.ndarray, table: np.ndarray) -> np.ndarray:
    from concourse.bass_utils import run_bass_kernel_spmd

    x = np.asarray(x)
    table = np.asarray(table)
    assert x.shape == (BATCH, XCOLS) and table.shape == (L, 2, 48, 8)
    nc = _get_module()
    res = run_bass_kernel_spmd(nc, make_in_maps(x, table), core_ids=list(range(NCORES)))
    return assemble_output([res.results[i]["out"] for i in range(NCORES)])


# revision 20
# speedup vs baseline: 1.0490x; 1.0392x over previous
"""Trainium2 Bass kernel for nn_Decoder: bit-unpack 23x22-bit codes per batch
row, gather 1536B fp16 table rows by index, sign-flip about 0.5, emit the data
rows (rows 19:67 of each [2, 126, 128] plane) as fp16. The constant-0.5 filler
regions and the fp32 upcast are data-independent, so the host materializes
them during unshard (assemble_output) — this cuts device HBM writes from
132MB to 25MB per core.

Gathers use per-code indirect DMAs: one offset per partition is the only
form the DMA_INDIRECT ucode supports (multi-offset APs stream contiguously
from offset[p,0] or crash the exec unit). At ~1.4us serialized descriptor-gen
per instruction on the GpSimd engine, the 184 gathers per core (8 groups x 23
codes) are the critical path (~260us of the ~290us exec).

Sharding: data-parallel over batch across 8 NeuronCores (1024 rows each);
the lookup table is replicated on every core.

Self-contained: hardcodes all shapes; no imports from the problem directory.
"""

import numpy as np

import concourse.bacc as bacc
import concourse.bass as bass
import concourse.mybir as mybir
import concourse.tile as tile

BATCH = 8192
XCOLS = 512
NCODE = 23
NBITS = 22
L = 131072
ROW = 2 * 48 * 8     # 768 fp16 = 1536B per table row
NCORES = 8
BC = BATCH // NCORES
P = 128
GROUPS = BC // P
DROW = 2 * 48 * 128  # 12288

f16 = mybir.dt.float16
f32 = mybir.dt.float32
i32 = mybir.dt.int32

N_SWDGE_QUEUES = 2


def _code_map(c):
    if c < 7:
        return 0, 4, c * 8
    if c < 14:
        return 4, 4, (c - 7) * 8 + 4
    return 0, 8, (c - 7) * 8


def build_module():
    nc = bacc.Bacc(
        "TRN2", target_bir_lowering=False, debug=False,
        num_swdge_queues=N_SWDGE_QUEUES,
    )
    x_t = nc.dram_tensor("x", [BC, XCOLS], i32, kind="ExternalInput")
    tab_t = nc.dram_tensor("table", [L, ROW], f16, kind="ExternalInput")
    w_t = nc.dram_tensor("w", [P, NCODE * NBITS], f32, kind="ExternalInput")
    out_t = nc.dram_tensor("out", [BC, DROW], f16, kind="ExternalOutput")

    with tile.TileContext(nc) as tc:
        with (
            tc.tile_pool(name="const", bufs=1) as cpool,
            tc.tile_pool(name="xp", bufs=2) as xpool,
            tc.tile_pool(name="sm", bufs=GROUPS) as spool,
            tc.tile_pool(name="gt", bufs=32) as gpool,
            tc.tile_pool(name="op", bufs=2) as opool,
        ):
            # Fast path for the very first gather: it only needs idx of
            # (group 0, code 0) = bits x[0:128, 6:28]. A 128B/row load plus a
            # 22-wide decode chain gets it ready ~4us before the full group-0
            # decode, pulling the whole gather stream earlier.
            NF = 4  # fast-path codes: covers gathers 0..3 until full decode lands
            x0a = cpool.tile([P, 6 + NF * NBITS], i32)
            nc.sync.dma_start(x0a[:], x_t[0:P, 0 : 6 + NF * NBITS])
            # dedicated weight slice so the fast path is not gated on the
            # full 506-element w load's completion
            w0_tile = cpool.tile([P, NF * NBITS], f32)
            nc.sync.dma_start(w0_tile[:], w_t[0:P, 0 : NF * NBITS])
            w_tile = cpool.tile([P, NCODE * NBITS], f32)
            nc.sync.dma_start(w_tile[:], w_t[:])
            xfa = cpool.tile([P, 6 + NF * NBITS], f32)
            nc.vector.tensor_copy(out=xfa[:], in_=x0a[:])
            proda = cpool.tile([P, NF * NBITS], f32)
            nc.vector.tensor_tensor(
                out=proda[:], in0=xfa[:, 6:], in1=w0_tile[:],
                op=mybir.AluOpType.mult,
            )
            codes0 = cpool.tile([P, NF], f32)
            nc.vector.tensor_reduce(
                out=codes0[:],
                in_=proda[:].rearrange("n (c a) -> n c a", a=NBITS),
                axis=mybir.AxisListType.X,
                op=mybir.AluOpType.add,
            )
            codes0i = cpool.tile([P, NF], i32)
            nc.vector.tensor_copy(out=codes0i[:], in_=codes0[:])
            idx0 = cpool.tile([P, NF], i32)
            nc.vector.tensor_scalar(
                out=idx0[:], in0=codes0i[:],
                scalar1=L - 1, scalar2=None,
                op0=mybir.AluOpType.bitwise_and,
            )

            idxs, tts, sgs = [], [], []
            for g in range(GROUPS):
                b0 = g * P
                x_tile = xpool.tile([P, XCOLS], i32)
                nc.sync.dma_start(x_tile[:], x_t[b0 : b0 + P, :])
                xf = xpool.tile([P, XCOLS], f32)
                nc.vector.tensor_copy(out=xf[:], in_=x_tile[:])
                prod = xpool.tile([P, NCODE * NBITS], f32)
                nc.vector.tensor_tensor(
                    out=prod[:], in0=xf[:, 6:], in1=w_tile[:],
                    op=mybir.AluOpType.mult,
                )
                codes = spool.tile([P, NCODE], f32, tag="codes")
                nc.vector.tensor_reduce(
                    out=codes[:],
                    in_=prod[:].rearrange("n (c a) -> n c a", a=NBITS),
                    axis=mybir.AxisListType.X,
                    op=mybir.AluOpType.add,
                )
                codesi = spool.tile([P, NCODE], i32, tag="codesi")
                nc.vector.tensor_copy(out=codesi[:], in_=codes[:])
                idx = spool.tile([P, NCODE], i32, tag="idx")
                nc.vector.tensor_scalar(
                    out=idx[:], in0=codesi[:],
                    scalar1=L - 1, scalar2=None,
                    op0=mybir.AluOpType.bitwise_and,
                )
                tt = spool.tile([P, NCODE], f32, tag="tt")
                nc.vector.tensor_scalar(
                    out=tt[:], in0=codes[:],
                    scalar1=float(L), scalar2=None,
                    op0=mybir.AluOpType.is_gt,
                )
                sg = spool.tile([P, NCODE], f32, tag="sg")
                nc.vector.tensor_scalar(
                    out=sg[:], in0=tt[:],
                    scalar1=-2.0, scalar2=1.0,
                    op0=mybir.AluOpType.mult, op1=mybir.AluOpType.add,
                )
                idxs.append(idx); tts.append(tt); sgs.append(sg)

            for g in range(GROUPS):
                b0 = g * P
                idx, tt, sg = idxs[g], tts[g], sgs[g]
                od = opool.tile([P, DROW], f16)
                od4 = od[:].rearrange("n (p k c) -> n p k c", p=2, k=48)
                # last group: wide codes first so the store waits on a fast
                # DVE consumer (~0.45us) instead of an ACT one (~1.1us)
                if g == GROUPS - 1:
                    code_order = list(range(14, NCODE)) + list(range(14))
                else:
                    code_order = list(range(NCODE))
                for c in code_order:
                    gc = gpool.tile([P, ROW], f16)
                    off = idx0[:, c : c + 1] if (g == 0 and c < NF) else idx[:, c : c + 1]
                    gi = nc.gpsimd.indirect_dma_start(
                        out=gc[:],
                        out_offset=None,
                        in_=tab_t[:],
                        in_offset=bass.IndirectOffsetOnAxis(ap=off, axis=0),
                    )
                    if N_SWDGE_QUEUES > 1 and c % N_SWDGE_QUEUES:
                        gi.ins.queue = f"qPoolDynamic{c % N_SWDGE_QUEUES}"
                    gv = gc[:].rearrange("n (p k c) -> n p k c", p=2, k=48)
                    ch0, wdt, col0 = _code_map(c)
                    if c >= 14:
                        nc.scalar.activation(
                            out=od4[:, :, :, col0 : col0 + wdt],
                            in_=gv[:, :, :, ch0 : ch0 + wdt],
                            func=mybir.ActivationFunctionType.Identity,
                            bias=tt[:, c : c + 1],
                            scale=sg[:, c : c + 1],
                        )
                    else:
                        nc.vector.tensor_scalar(
                            out=od4[:, :, :, col0 : col0 + wdt],
                            in0=gv[:, :, :, ch0 : ch0 + wdt],
                            scalar1=sg[:, c : c + 1],
                            scalar2=tt[:, c : c + 1],
                            op0=mybir.AluOpType.mult,
                            op1=mybir.AluOpType.add,
                        )
                eng = nc.sync if g % 2 == 0 else nc.scalar
                eng.dma_start(out=out_t[b0 : b0 + P, :], in_=od[:])
    nc.compile()
    return nc


def make_weights():
    w = np.tile((2.0 ** np.arange(NBITS)).astype(np.float32), NCODE)
    return np.broadcast_to(w, (P, NCODE * NBITS)).copy()


def make_in_maps(x, table):
    tab = np.ascontiguousarray(table.reshape(L, ROW))
    w = make_weights()
    return [
        {
            "x": np.ascontiguousarray(x[i * BC : (i + 1) * BC]),
            "table": tab,
            "w": w,
        }
        for i in range(NCORES)
    ]


def assemble_output(parts):
    out = np.full((BATCH, 2, 126, 128), 0.5, dtype=np.float32)
    for i, p in enumerate(parts):
        out[i * BC : (i + 1) * BC, :, 19:67, :] = p.reshape(BC, 2, 48, 128)
    return out


_NC_CACHE = None


def _get_module():
    global _NC_CACHE
    if _NC_CACHE is None:
        _NC_CACHE = build_module()
    return _NC_CACHE


def kernel(x: np.ndarray, table: np.ndarray) -> np.ndarray:
    from concourse.bass_utils import run_bass_kernel_spmd

    x = np.asarray(x)
    table = np.asarray(table)
    assert x.shape == (BATCH, XCOLS) and table.shape == (L, 2, 48, 8)
    nc = _get_module()
    res = run_bass_kernel_spmd(nc, make_in_maps(x, table), core_ids=list(range(NCORES)))
    return assemble_output([res.results[i]["out"] for i in range(NCORES)])
